# revision 13
# baseline (speedup 1.0000x reference)
"""Quantized int8 conv2d (brevitas-style) on 8 TRN2 NeuronCores.

Data-parallel over batch (1 image / core). Per-tensor symmetric int8
quantization: local abs-max -> AllReduce(max) -> quantize -> 3x3 conv
(stride 1, pad 1) as PE col-tiled matmuls -> dequant + bias.

Key tricks:
- x is cached in SBUF as fp16 during the abs-max pass (single DRAM read).
- round(v) is computed as fp16(v + 1536) (fp16 RNE at the [1024,2048)
  binade has ulp=1 -> exact round-half-even, matching jnp.round). The
  +1536 offset rides through the conv matmuls and is cancelled exactly
  by a correction matmul per output row group using {0,1536} column
  patterns that replicate the zero-padding tap structure.
- Conv: output rows grouped by 4 (c = h%4). Each c is one PE column
  tile (0, 32c), all four concurrent, each accumulating into its own
  PSUM bank: 3 K=128 matmuls (kw taps; lhsT has zero row-blocks where
  the input row class is invalid) + 1 correction + (c=0/c=3) 3 windowed
  K=32 boundary matmuls against a staged copy of the adjacent block's
  edge row.
"""

import sys

if "/opt/trn_rl_repo" not in sys.path:
    sys.path.insert(0, "/opt/trn_rl_repo")

import numpy as np

import concourse.bass as bass
import concourse.bacc as bacc
import concourse.mybir as mybir
from concourse import tile
from concourse.bass_utils import run_bass_kernel_spmd

N_CORES = 8
C = 32
O = 32
H = 512
W = 512
F32 = mybir.dt.float32
F16 = mybir.dt.float16

MAXV = 127.0
RND = 1536.0

# per-kw output/rhs column windows: (out_start, rhs_start, n)
KW_COLS = {0: (1, 0, 511), 1: (0, 0, 512), 2: (0, 1, 511)}
G = 4  # q-blocks per DMA group


def build_nc(h=H):
    nc = bacc.Bacc(None, target_bir_lowering=False, debug=False)
    NQ = h // 4
    NG = NQ // G

    x_ext = nc.declare_dram_parameter("x", [C, h, W], F32, isOutput=False)
    w_ext = nc.declare_dram_parameter("weight", [O, C, 3, 3], F32, isOutput=False)
    b_ext = nc.declare_dram_parameter("bias", [O], F32, isOutput=False)
    out_ext = nc.declare_dram_parameter("out", [O, h, W], F32, isOutput=True)

    cc_in = nc.dram_tensor("cc_in", [128], F32)
    cc_out = nc.dram_tensor("cc_out", [128], F32, addr_space="Shared")

    with tile.TileContext(nc) as tc:
        with (
            tc.tile_pool(name="persist", bufs=1) as persist,
            tc.tile_pool(name="stage", bufs=3) as stage,
            tc.tile_pool(name="qx", bufs=6) as qxp,
            tc.tile_pool(name="outp", bufs=3) as outp,
            tc.tile_pool(name="spp", bufs=3) as spp,
            tc.tile_pool(name="snp", bufs=3) as snp,
            tc.tile_pool(name="ps0", bufs=2, space="PSUM") as psp0,
            tc.tile_pool(name="ps1", bufs=2, space="PSUM") as psp1,
            tc.tile_pool(name="ps2", bufs=2, space="PSUM") as psp2,
            tc.tile_pool(name="ps3", bufs=2, space="PSUM") as psp3,
        ):
            psps = [psp0, psp1, psp2, psp3]
            # ---------------- persistent SBUF tensors ----------------
            x4 = persist.tile([128, NQ * W], F16)
            maxes = persist.tile([128, NQ], F32)
            wsb = persist.tile([128, 288], F32)
            qw = persist.tile([128, 288], F16)
            tq = persist.tile([128, 288], F16)
            cw = persist.tile([128, 288], F16)
            qwc = persist.tile([128, 384], F16)   # main lhsT (c,kw) blocks
            cw4 = persist.tile([96, 96], F16)     # corr lhsT rows (kw,i)
            cb4 = persist.tile([96, W], F16)      # corr rhs patterns
            ones_l = persist.tile([1, 128], F32)
            bias_sb = persist.tile([128, 1], F32)
            gmax = persist.tile([128, 1], F32)
            gmax2 = persist.tile([128, 1], F32)
            gmaxr = persist.tile([1, 128], F32)
            wred = persist.tile([128, 1], F32)
            wredr = persist.tile([1, 128], F32)
            sg = persist.tile([1, 1], F32)
            sw = persist.tile([1, 1], F32)
            inv = persist.tile([1, 1], F32)
            invw = persist.tile([1, 1], F32)
            cwi = persist.tile([1, 1], F32)
            cqi = persist.tile([1, 1], F32)
            dqi = persist.tile([1, 1], F32)
            bc_in = persist.tile([1, 4], F32)
            bvec = persist.tile([128, 4], F32)
            cw_ap = persist.tile([128, 1], F32)
            s01 = persist.tile([128, 96], F16)
            s12 = persist.tile([128, 96], F16)
            sall = persist.tile([128, 96], F16)

            # ---------------- weight path (local) --------------------
            wv = w_ext[:, :, :, :].rearrange("o i kh kw -> i kh kw o")
            for g in range(4):
                nc.sync.dma_start(out=wsb[32 * g : 32 * g + 32, :], in_=wv)
            for cix in range(4):
                nc.sync.dma_start(
                    out=bias_sb[32 * cix : 32 * cix + 32, :], in_=b_ext[:, None]
                )
            nc.gpsimd.memset(ones_l[:, :], 1.0)
            nc.gpsimd.memset(qwc[:, :], 0.0)
            nc.gpsimd.memset(cb4[:, :], RND)
            nc.gpsimd.memset(cb4[0:32, 0:1], 0.0)
            nc.gpsimd.memset(cb4[64:96, W - 1 : W], 0.0)

            # sw = max |w| (X-reduce, fold partitions to a row, reduce again)
            nc.vector.tensor_reduce(
                out=wred[:, :], in_=wsb[:, :], axis=mybir.AxisListType.X,
                op=mybir.AluOpType.max, apply_absolute_value=True,
            )
            nc.sync.dma_start(out=wredr[0:1, 0:128], in_=wred[:, 0:1])
            nc.vector.tensor_reduce(
                out=sw[:, :], in_=wredr[:, :], axis=mybir.AxisListType.X,
                op=mybir.AluOpType.max,
            )
            nc.vector.reciprocal(invw[:, :], sw[:, :])
            nc.vector.tensor_scalar_mul(cwi[:, :], invw[:, :], MAXV)

            if True:
                bps = psp0.tile([128, 4], F32, tag="pst0")
                nc.tensor.matmul(bps[:, 0:1], ones_l[:, :], cwi[:, :])
                nc.vector.tensor_copy(cw_ap[:, :], bps[:, 0:1])

                # qw = round(w * 127/sw) via fp16 +1536 trick
                nc.scalar.activation(
                    out=tq[:, :], in_=wsb[:, :],
                    func=mybir.ActivationFunctionType.Copy,
                    scale=cw_ap[:, 0:1], bias=RND,
                )
                with nc.allow_low_precision("int8 values exact in fp16"):
                    nc.vector.tensor_scalar_add(qw[:, :], tq[:, :], -RND)
                    nc.vector.tensor_add(s01[:, :], qw[:, 0:96], qw[:, 96:192])
                    nc.vector.tensor_add(s12[:, :], qw[:, 96:192], qw[:, 192:288])
                    nc.vector.tensor_add(sall[:, :], s01[:, :], qw[:, 192:288])
                    nc.vector.tensor_scalar_mul(cw[:, 0:96], sall[:, :], -1.0)
                    nc.vector.tensor_scalar_mul(cw[:, 96:192], s12[:, :], -1.0)
                    nc.vector.tensor_scalar_mul(cw[:, 192:288], s01[:, :], -1.0)
                    # main lhsT: qwc[32*hm+i, (c*3+kw)*32+o] = qw[o,i,hm-c+1,kw]
                    for cix in range(4):
                        for kw in range(3):
                            for kh in range(3):
                                hm = cix + kh - 1
                                if not (0 <= hm <= 3):
                                    continue
                                nc.vector.tensor_copy(
                                    qwc[32 * hm : 32 * hm + 32,
                                        (cix * 3 + kw) * 32 : (cix * 3 + kw) * 32 + 32],
                                    qw[0:32, kh * 96 + kw * 32 : kh * 96 + kw * 32 + 32],
                                )
                    for v in range(3):
                        for kw in range(3):
                            nc.vector.tensor_copy(
                                cw4[32 * kw : 32 * kw + 32, v * 32 : v * 32 + 32],
                                cw[0:32, v * 96 + kw * 32 : v * 96 + kw * 32 + 32],
                            )

                # ------------- pass 1: stream x, absmax + fp16 cache --
                for q in range(NQ):
                    stg = stage.tile([128, W], F32)
                    xv = x_ext[:, 4 * q : 4 * q + 4, :].rearrange("i hm w -> hm i w")
                    nc.sync.dma_start(out=stg[:, :], in_=xv)
                    nc.scalar.activation(
                        out=x4[:, q * W : (q + 1) * W], in_=stg[:, :],
                        func=mybir.ActivationFunctionType.Copy,
                    )
                    nc.vector.tensor_reduce(
                        out=maxes[:, q : q + 1], in_=stg[:, :],
                        axis=mybir.AxisListType.X,
                        op=mybir.AluOpType.max, apply_absolute_value=True,
                    )

                nc.vector.tensor_reduce(
                    out=gmax[:, :], in_=maxes[:, :], axis=mybir.AxisListType.X,
                    op=mybir.AluOpType.max,
                )

                # ------------- all-reduce(max) across 8 cores ---------
                nc.sync.dma_start(out=cc_in[:, None], in_=gmax[:, :])
                nc.gpsimd.collective_compute(
                    "AllReduce", mybir.AluOpType.max,
                    replica_groups=[list(range(N_CORES))],
                    ins=[cc_in[:].opt()], outs=[cc_out[:].opt()],
                )
                nc.sync.dma_start(out=gmax2[:, :], in_=cc_out[:, None])
                nc.sync.dma_start(out=gmaxr[0:1, 0:128], in_=gmax2[:, 0:1])
                nc.vector.tensor_reduce(
                    out=sg[:, :], in_=gmaxr[:, :], axis=mybir.AxisListType.X,
                    op=mybir.AluOpType.max,
                )

                nc.vector.reciprocal(inv[:, :], sg[:, :])
                nc.vector.tensor_scalar_mul(cqi[:, :], inv[:, :], MAXV)
                nc.vector.tensor_mul(dqi[:, :], sg[:, :], sw[:, :])
                nc.vector.tensor_scalar_mul(dqi[:, :], dqi[:, :], 1.0 / (MAXV * MAXV))
                nc.vector.tensor_copy(bc_in[:, 0:1], cqi[:, :])
                nc.vector.tensor_copy(bc_in[:, 1:2], dqi[:, :])
                bps2 = psp1.tile([128, 4], F32, tag="pst1")
                nc.tensor.matmul(bps2[:, 0:2], ones_l[:, :], bc_in[:, 0:2])
                nc.vector.tensor_copy(bvec[:, 0:2], bps2[:, 0:2])
            cq_ap = bvec[:, 0:1]
            dq_ap = bvec[:, 1:2]

            # ---------------- pass 2 ----------------------------------
            qx_tiles = {}

            def quantize_block(j):
                t = qxp.tile([128, W], F16)
                nc.scalar.activation(
                    out=t[:, :], in_=x4[:, j * W : (j + 1) * W],
                    func=mybir.ActivationFunctionType.Copy,
                    scale=cq_ap, bias=RND,
                )
                qx_tiles[j] = t

            quantize_block(0)
            quantize_block(1)

            ot4 = None
            for q in range(NQ):
                if q + 2 <= NQ - 1:
                    quantize_block(q + 2)

                sp32 = sn32 = None
                if q > 0:
                    sp32 = spp.tile([32, W], F16)
                    nc.gpsimd.dma_start(out=sp32[:, :], in_=qx_tiles[q - 1][96:128, :])
                if q < NQ - 1:
                    sn32 = snp.tile([32, W], F16)
                    nc.gpsimd.dma_start(out=sn32[:, :], in_=qx_tiles[q + 1][0:32, :])

                cur = qx_tiles[q]
                pss = []
                for cix in range(4):
                    pst = psps[cix].tile([128, W], F32, tag=f"pst{cix}")
                    pss.append(pst)
                    mms = []
                    for kw in (1, 0, 2):
                        oc0, rc0, nn = KW_COLS[kw]
                        mms.append(
                            (qwc[0:128, (cix * 3 + kw) * 32 : (cix * 3 + kw) * 32 + 32],
                             cur[0:128, rc0 : rc0 + nn],
                             pst[32 * cix : 32 * cix + 32, oc0 : oc0 + nn])
                        )
                    v = 1 if (q == 0 and cix == 0) else (
                        2 if (q == NQ - 1 and cix == 3) else 0)
                    mms.append(
                        (cw4[0:96, v * 32 : v * 32 + 32], cb4[0:96, 0:W],
                         pst[32 * cix : 32 * cix + 32, 0:W])
                    )
                    if cix == 0 and sp32 is not None:
                        for kw in (1, 0, 2):
                            oc0, rc0, nn = KW_COLS[kw]
                            mms.append(
                                (qw[0:32, kw * 32 : kw * 32 + 32],  # kh=0
                                 sp32[0:32, rc0 : rc0 + nn],
                                 pst[0:32, oc0 : oc0 + nn])
                            )
                    if cix == 3 and sn32 is not None:
                        for kw in (1, 0, 2):
                            oc0, rc0, nn = KW_COLS[kw]
                            mms.append(
                                (qw[0:32, 192 + kw * 32 : 192 + kw * 32 + 32],  # kh=2
                                 sn32[0:32, rc0 : rc0 + nn],
                                 pst[96:128, oc0 : oc0 + nn])
                            )
                    for mi, (lhsT, rhs, outap) in enumerate(mms):
                        nc.tensor.matmul(
                            outap, lhsT, rhs,
                            start=(mi == 0), stop=(mi == len(mms) - 1),
                            tile_position=(0, 32 * cix),
                        )

                ot4 = outp.tile([128, W], F32)
                qo = 0
                for cix in range(4):
                    sl = slice(32 * cix, 32 * cix + 32)
                    if cix % 2 == 0:
                        nc.vector.tensor_scalar(
                            out=ot4[sl, qo : qo + W],
                            in0=pss[cix][sl, :],
                            scalar1=dq_ap[sl, :],
                            scalar2=bias_sb[sl, :],
                            op0=mybir.AluOpType.mult, op1=mybir.AluOpType.add,
                        )
                    else:
                        nc.scalar.activation(
                            out=ot4[sl, qo : qo + W],
                            in_=pss[cix][sl, :],
                            func=mybir.ActivationFunctionType.Identity,
                            scale=dq_ap[sl, :],
                            bias=bias_sb[sl, :],
                        )

                ov = out_ext[:, 4 * q : 4 * q + 4, :].rearrange("o hm w -> hm o w")
                nc.sync.dma_start(out=ov, in_=ot4[:, :])

    nc.finalize()
    return nc


_NC_CACHE = {}


def kernel(x, weight, bias):
    x = np.ascontiguousarray(x, dtype=np.float32)
    weight = np.ascontiguousarray(weight, dtype=np.float32)
    bias = np.ascontiguousarray(bias, dtype=np.float32)
    if "nc" not in _NC_CACHE:
        _NC_CACHE["nc"] = build_nc()
    nc = _NC_CACHE["nc"]
    in_maps = [
        {"x": x[i], "weight": weight, "bias": bias} for i in range(N_CORES)
    ]
    res = run_bass_kernel_spmd(nc, in_maps, core_ids=list(range(N_CORES)))
    outs = [res.results[i]["out"] for i in range(N_CORES)]
    return np.stack(outs, axis=0)


if __name__ == "__main__":
    build_nc(h=32)
    print("build ok")
